# revision 5
# baseline (speedup 1.0000x reference)
"""Depthwise 5x5 SAME conv (B=16, H=W=512, C=8) on 8 TRN2 NeuronCores — v2.

Strategy (data-parallel over batch, 2 images per core), all bf16 on-wire
(rel-err budget 2e-2; bf16 in/weights/out measures ~2.9e-3):

  - Host casts to bf16 and lays out channel-planar overlapped H-blocks:
    4 main blocks of 128 input rows (in-rows 124t-2 .. 124t+126, zero-padded
    at H edges) -> 124 output rows each (0..495), plus a rump block covering
    out rows 496..511 packed (c,h) into partitions.
  - Free-dim layout per partition row: (c, img, wp) with wp = 516 (W padded
    by 2 each side). One 2.06 MB DMA per block (both images).
  - Conv: per (block, c, img): 5 matmuls (one per kw tap j, ISA moving max
    is 512 elems) accumulating into a 2-bank PSUM tile [124, 1024]; lhsT =
    [128, 124] banded matrix (5 kh taps on diagonals).
  - Rump: (4c x 20 in-rows) = 80 partitions x 2 groups; banded block-diag
    lhsT [80, 64] -> 16 out rows x 4c per group. Computed first: its tiny
    loads + 20 matmuls mask the block-0 load.
  - Evac PSUM f32 -> bf16 with bias, alternating ScalarE/VectorE.
  - Loads on sync (SP HWDGE queue); constants + stores on scalar (ACT
    queue). All big DMAs are [128 partitions x contiguous DRAM] - partial
    partition counts or strided DRAM halve DMA throughput. Block 0 loads
    per-channel (DRAM chunked) so channel 0 matmuls start ~1us in; stores
    go out as two [128, 4096] halves per block.
"""
import os
import sys

for _p in ("/opt/trn_rl_repo",):
    if _p not in sys.path and os.path.isdir(_p):
        sys.path.insert(0, _p)

import numpy as np
import ml_dtypes

BF16 = ml_dtypes.bfloat16

B, H, W, C = 16, 512, 512, 8
KH = KW = 5
PAD = 2
WP = W + 2 * PAD            # 516
N_CORES = 8
B_PER_CORE = B // N_CORES   # 2
MT = 124                    # out rows per main block
NT = 4                      # main blocks
RUMP_OUT = H - NT * MT      # 16
RUMP_IN = RUMP_OUT + 4      # 20 in rows per channel
FREE_X = C * B_PER_CORE * WP      # 8256
FREE_Y = C * B_PER_CORE * W       # 8192
NW = B_PER_CORE * W               # 1024 matmul free size

_PROG = None
LAST_EXEC_NS = None


def _build_program(reps=1, mode="full"):
    import concourse.bacc as bacc
    import concourse.tile as tile
    from concourse import mybir

    f32 = mybir.dt.float32
    bf16 = mybir.dt.bfloat16
    IDENT = mybir.ActivationFunctionType.Identity

    nc = bacc.Bacc()
    xm_d = nc.dram_tensor("xm", [NT * 128, FREE_X], bf16, kind="ExternalInput")
    xr_d = nc.dram_tensor("xr", [2, 80, B_PER_CORE * WP], bf16,
                          kind="ExternalInput")
    bands_d = nc.dram_tensor("bands", [128, C * KW * MT], bf16,
                             kind="ExternalInput")
    sbands_d = nc.dram_tensor("sbands", [80, 2 * KW * 64], bf16,
                              kind="ExternalInput")
    bias_d = nc.dram_tensor("bias128", [128, C], f32, kind="ExternalInput")
    biasr_d = nc.dram_tensor("biasr", [128, 1], f32, kind="ExternalInput")
    ym_d = nc.dram_tensor("ym", [(NT - 1) * 2 * 128, FREE_Y // 2], bf16,
                          kind="ExternalOutput")
    ym3_d = nc.dram_tensor("ym3", [4 * 128, FREE_Y // 4], bf16,
                           kind="ExternalOutput")
    yr_d = nc.dram_tensor("yr", [128, NW], bf16, kind="ExternalOutput")

    with tile.TileContext(nc) as tc:
        with (
            tc.tile_pool(name="wp", bufs=1) as wp,
            tc.tile_pool(name="xp", bufs=4) as xp,
            tc.tile_pool(name="op", bufs=3) as op_,
            tc.tile_pool(name="rp", bufs=1) as rp,
            tc.tile_pool(name="pp", bufs=4, space="PSUM") as pp,
        ):
            def loop_body():
                # constants ride the ACT queue (store queue idle at
                # start); rump deps (sbands/biasr) lead so rump compute can
                # run during the main block loads
                xrts = []
                for g in range(2):
                    xrt = rp.tile([80, B_PER_CORE * WP], bf16, tag=f"xr{g}",
                                  name=f"xr_{g}")
                    eng = nc.sync if g == 0 else nc.scalar
                    eng.dma_start(out=xrt, in_=xr_d[g, :, :])
                    xrts.append(xrt)
                sbands = wp.tile([80, 2 * KW * 64], bf16, tag="sbands")
                nc.scalar.dma_start(out=sbands, in_=sbands_d[:, :])
                biasrt = wp.tile([128, 1], f32, tag="biasr")
                nc.scalar.dma_start(out=biasrt, in_=biasr_d[:, :])
                biast = wp.tile([128, C], f32, tag="bias")
                nc.scalar.dma_start(out=biast, in_=bias_d[:, :])
                bands = wp.tile([128, C * KW * MT], bf16, tag="bands")
                nc.scalar.dma_start(out=bands, in_=bands_d[:, :])

                # block 0 is laid out (c, p, img, wp) in DRAM: per-channel
                # chunks load as [128, 1032] contiguous pieces so channel 0's
                # matmuls can start ~1us in instead of waiting for 2 MB
                xm_c = xm_d.ap().rearrange("r (b w) -> (r b) w", b=C)
                CWB = B_PER_CORE * WP
                xts = []
                for t in range(NT):
                    xt = xp.tile([128, FREE_X], bf16, tag="x",
                                 name=f"x_{t}")
                    if t == 0:
                        for cc in range(C):
                            nc.sync.dma_start(
                                out=xt[:, cc * CWB:(cc + 1) * CWB],
                                in_=xm_c[cc * 128:(cc + 1) * 128, :])
                    else:
                        nc.sync.dma_start(out=xt,
                                          in_=xm_d[128 * t:128 * (t + 1), :])
                    xts.append(xt)

                def evac(idx, psum_t, out_view, bias_ap):
                    if idx % 2 == 0:
                        nc.scalar.activation(out=out_view, in_=psum_t,
                                             func=IDENT, bias=bias_ap,
                                             scale=1.0)
                    else:
                        nc.vector.tensor_scalar_add(out=out_view, in0=psum_t,
                                                    scalar1=bias_ap)

                # rump first: tiny loads + 20 matmuls warm the PE while
                # the big block loads stream in; yr store retires early
                if mode != "dma":
                    ort = rp.tile([128, NW], bf16, tag="or")
                    for g in range(2):
                        pr = pp.tile([64, NW], f32, tag="ps", name=f"ps_r{g}")
                        xrv = xrts[g][:, :].rearrange("p (i w) -> p i w",
                                                      i=B_PER_CORE)
                        for j in range(KW):
                            lhsT = sbands[:, (g * KW + j) * 64:
                                          (g * KW + j + 1) * 64]
                            for img in range(B_PER_CORE):
                                nc.tensor.matmul(
                                    pr[:, img * W:(img + 1) * W], lhsT,
                                    xrv[:, img, j:j + W],
                                    start=(j == 0), stop=(j == KW - 1))
                        if mode != "pe":
                            evac(g, pr, ort[g * 64:(g + 1) * 64, :],
                                 biasrt[g * 64:(g + 1) * 64, 0:1])
                    if mode == "full":
                        nc.scalar.dma_start(out=yr_d[:, :], in_=ort[:, :])

                for t in range(NT):
                    if mode == "dma":
                        if t < NT - 1:
                            for hh in range(2):
                                r0 = (2 * t + hh) * 128
                                nc.scalar.dma_start(
                                    out=ym_d[r0:r0 + 128, :],
                                    in_=xts[t][:, hh * (FREE_Y // 2):
                                               (hh + 1) * (FREE_Y // 2)])
                        else:
                            for qq in range(4):
                                nc.scalar.dma_start(
                                    out=ym3_d[qq * 128:(qq + 1) * 128, :],
                                    in_=xts[t][:, qq * (FREE_Y // 4):
                                               (qq + 1) * (FREE_Y // 4)])
                        continue
                    ot = op_.tile([128, FREE_Y], bf16, tag="o", name=f"o_{t}")
                    # [128, (c, img, wp)] view for strided rhs windows
                    xv = xts[t][:, :].rearrange("p (c i w) -> p c i w",
                                                c=C, i=B_PER_CORE)
                    for c in range(C):
                        pt = pp.tile([MT, NW], f32, tag="ps",
                                     name=f"ps_{t}_{c}")
                        for j in range(KW):
                            lhsT = bands[:, (c * KW + j) * MT:
                                         (c * KW + j + 1) * MT]
                            for img in range(B_PER_CORE):
                                nc.tensor.matmul(
                                    pt[:, img * W:(img + 1) * W], lhsT,
                                    xv[:, c, img, j:j + W],
                                    start=(j == 0), stop=(j == KW - 1))
                        if mode != "pe":
                            evac(c, pt, ot[0:MT, c * NW:(c + 1) * NW],
                                 biast[0:MT, c:c + 1])
                    if mode == "full":
                        if t < NT - 1:
                            for hh in range(2):
                                r0 = (2 * t + hh) * 128
                                nc.scalar.dma_start(
                                    out=ym_d[r0:r0 + 128, :],
                                    in_=ot[:, hh * (FREE_Y // 2):
                                           (hh + 1) * (FREE_Y // 2)])
                        else:
                            # last block: 4 quarter-stores on both queues so
                            # the drain tail is one 512 KB transfer
                            for qq in range(4):
                                eng = nc.scalar if qq % 2 == 0 else nc.sync
                                eng.dma_start(
                                    out=ym3_d[qq * 128:(qq + 1) * 128, :],
                                    in_=ot[:, qq * (FREE_Y // 4):
                                           (qq + 1) * (FREE_Y // 4)])

            if reps == 1:
                loop_body()
            else:
                with tc.For_i(0, reps, 1, hint_engines=(mybir.EngineType.PE,)):
                    loop_body()

    nc.compile()
    return nc


def _make_bands(K):
    """K: [5,5,C] (kh, kw, c) f32. Returns bf16 band matrices."""
    bands = np.zeros((128, C * KW * MT), np.float32)
    q = np.arange(MT)
    for c in range(C):
        for j in range(KW):
            off = (c * KW + j) * MT
            for d in range(KH):
                bands[q + d, off + q] = K[d, j, c]
    sbands = np.zeros((80, 2 * KW * 64), np.float32)
    ho = np.arange(RUMP_OUT)
    for g in range(2):
        for j in range(KW):
            off = (g * KW + j) * 64
            for ci in range(4):
                cc = 4 * g + ci
                for d in range(KH):
                    sbands[ci * RUMP_IN + ho + d, off + ci * RUMP_OUT + ho] = \
                        K[d, j, cc]
    return bands.astype(BF16), sbands.astype(BF16)


def prepare_in_maps(x, K, bias):
    """x [B,H,W,C] f32, K [5,5,C], bias [C] -> per-core input dicts."""
    bands, sbands = _make_bands(K)
    bias128 = np.tile(bias[None, :], (128, 1)).astype(np.float32)
    biasr = np.empty((128, 1), np.float32)
    for g in range(2):
        for ci in range(4):
            biasr[g * 64 + ci * RUMP_OUT:g * 64 + (ci + 1) * RUMP_OUT, 0] = \
                bias[4 * g + ci]

    xb = x.astype(BF16)
    in_maps = []
    for i in range(N_CORES):
        xc = xb[i * B_PER_CORE:(i + 1) * B_PER_CORE]   # [2, 512, 512, 8]
        xh = np.zeros((B_PER_CORE, H + 4, C, WP), BF16)
        xh[:, 2:2 + H, :, PAD:PAD + W] = np.transpose(xc, (0, 1, 3, 2))
        xht = np.ascontiguousarray(xh.transpose(1, 2, 0, 3))  # [r, c, i, wp]
        xm = np.empty((NT * 128, C, B_PER_CORE, WP), BF16)
        for t in range(NT):
            if t == 0:
                xm[0:128] = np.ascontiguousarray(
                    xht[0:128].transpose(1, 0, 2, 3)).reshape(
                        128, C, B_PER_CORE, WP)
            else:
                xm[128 * t:128 * (t + 1)] = xht[MT * t:MT * t + 128]
        xr = np.empty((2, 80, B_PER_CORE * WP), BF16)
        for g in range(2):
            for ci in range(4):
                xr[g, ci * RUMP_IN:(ci + 1) * RUMP_IN] = \
                    xht[NT * MT:NT * MT + RUMP_IN, 4 * g + ci].reshape(
                        RUMP_IN, B_PER_CORE * WP)
        in_maps.append({
            "xm": xm.reshape(NT * 128, FREE_X),
            "xr": xr,
            "bands": bands,
            "sbands": sbands,
            "bias128": bias128,
            "biasr": biasr,
        })
    return in_maps


def assemble_output(results):
    """Per-core {ym, yr} bf16 -> full [B,H,W,C] f32."""
    out = np.empty((B, H, W, C), np.float32)
    for i in range(N_CORES):
        ym = np.asarray(results[i]["ym"]).reshape(NT - 1, 2, 128, C // 2,
                                                   B_PER_CORE, W)
        ym = ym.transpose(0, 2, 1, 3, 4, 5).reshape(NT - 1, 128, C,
                                                    B_PER_CORE, W)
        ym3 = np.asarray(results[i]["ym3"]).reshape(4, 128, C // 4,
                                                    B_PER_CORE, W)
        ym3 = ym3.transpose(1, 0, 2, 3, 4).reshape(1, 128, C, B_PER_CORE, W)
        ym = np.concatenate([ym, ym3], axis=0)
        ym = ym[:, :MT].reshape(NT * MT, C, B_PER_CORE, W)
        yr = np.asarray(results[i]["yr"]).reshape(2, 4, RUMP_OUT, B_PER_CORE, W)
        sl = slice(i * B_PER_CORE, (i + 1) * B_PER_CORE)
        # ym[h, c, img, w] -> [img, h, w, c]
        out[sl, :NT * MT] = ym.transpose(2, 0, 3, 1).astype(np.float32)
        # yr[g, ci, ho, img, w] -> [img, ho, w, (g ci)]
        out[sl, NT * MT:] = yr.transpose(3, 2, 4, 0, 1).reshape(
            B_PER_CORE, RUMP_OUT, W, C).astype(np.float32)
    return out


def kernel(x, kernel, bias):
    global _PROG, LAST_EXEC_NS
    from concourse.bass_utils import run_bass_kernel_spmd

    x = np.asarray(x, dtype=np.float32)
    K = np.asarray(kernel, dtype=np.float32).reshape(KH, KW, C)
    bias = np.asarray(bias, dtype=np.float32).reshape(C)

    if _PROG is None:
        _PROG = _build_program()

    in_maps = prepare_in_maps(x, K, bias)
    trace = os.environ.get("KERNEL_TRACE") == "1"
    res = run_bass_kernel_spmd(_PROG, in_maps, list(range(N_CORES)),
                               trace=trace)
    LAST_EXEC_NS = res.exec_time_ns
    if trace and res.exec_time_ns is not None:
        print(f"HW exec time: {res.exec_time_ns} ns")
    return assemble_output(res.results)


# revision 6
# speedup vs baseline: 1.0376x; 1.0376x over previous
"""Depthwise 5x5 SAME conv (B=16, H=W=512, C=8) on 8 TRN2 NeuronCores — v2.

Strategy (data-parallel over batch, 2 images per core), all bf16 on-wire
(rel-err budget 2e-2; bf16 in/weights/out measures ~2.9e-3):

  - Host casts to bf16 and lays out channel-planar overlapped H-blocks:
    4 main blocks of 128 input rows (in-rows 124t-2 .. 124t+126, zero-padded
    at H edges) -> 124 output rows each (0..495), plus a rump block covering
    out rows 496..511 packed (c,h) into partitions.
  - Free-dim layout per partition row: (c, img, wp) with wp = 516 (W padded
    by 2 each side). One 2.06 MB DMA per block (both images).
  - Conv: per (block, c, img): 5 matmuls (one per kw tap j, ISA moving max
    is 512 elems) accumulating into a 2-bank PSUM tile [124, 1024]; lhsT =
    [128, 124] banded matrix (5 kh taps on diagonals).
  - Rump: (4c x 20 in-rows) = 80 partitions x 2 groups; banded block-diag
    lhsT [80, 64] -> 16 out rows x 4c per group. Computed first: its tiny
    loads + 20 matmuls mask the block-0 load.
  - Evac PSUM f32 -> bf16 with bias, alternating ScalarE/VectorE.
  - Loads on sync (SP HWDGE queue); constants + stores on scalar (ACT
    queue). All big DMAs are [128 partitions x contiguous DRAM] - partial
    partition counts or strided DRAM halve DMA throughput. Block 0 loads
    per-channel (DRAM chunked) so channel 0 matmuls start ~1us in; stores
    go out as two [128, 4096] halves per block.
"""
import os
import sys

for _p in ("/opt/trn_rl_repo",):
    if _p not in sys.path and os.path.isdir(_p):
        sys.path.insert(0, _p)

import numpy as np
import ml_dtypes

BF16 = ml_dtypes.bfloat16

B, H, W, C = 16, 512, 512, 8
KH = KW = 5
PAD = 2
WP = W + 2 * PAD            # 516
N_CORES = 8
B_PER_CORE = B // N_CORES   # 2
NT = 4                      # main blocks
MTS = [126, 124, 124, 126]  # out rows per block (edge blocks use clipped
                            # bands: out-of-range taps hit H-edge zeros)
OBASE = [0, 126, 250, 386]  # first out row per block
RSTART = [0, 124, 248, 384]  # first in row per block (128 rows each)
RUMP_OUT = 12               # interior gap rows 374..385
RUMP_IN = 16                # in rows 372..387 per channel
FREE_X = C * B_PER_CORE * WP      # 8256
FREE_Y = C * B_PER_CORE * W       # 8192
NW = B_PER_CORE * W               # 1024 matmul free size

_PROG = None
LAST_EXEC_NS = None


def _build_program(reps=1, mode="full"):
    import concourse.bacc as bacc
    import concourse.tile as tile
    from concourse import mybir

    f32 = mybir.dt.float32
    bf16 = mybir.dt.bfloat16
    IDENT = mybir.ActivationFunctionType.Identity

    nc = bacc.Bacc()
    xm_d = nc.dram_tensor("xm", [NT * 128, FREE_X], bf16, kind="ExternalInput")
    xr_d = nc.dram_tensor("xr", [128, B_PER_CORE * WP], bf16,
                          kind="ExternalInput")
    bands0_d = nc.dram_tensor("bands0", [128, C * KW * 126], bf16,
                              kind="ExternalInput")
    bands_d = nc.dram_tensor("bands", [128, C * KW * 124], bf16,
                             kind="ExternalInput")
    bands3_d = nc.dram_tensor("bands3", [128, C * KW * 126], bf16,
                              kind="ExternalInput")
    sbands_d = nc.dram_tensor("sbands", [128, KW * C * RUMP_OUT], bf16,
                              kind="ExternalInput")
    bias_d = nc.dram_tensor("bias128", [128, C], f32, kind="ExternalInput")
    biasr_d = nc.dram_tensor("biasr", [128, 1], f32, kind="ExternalInput")
    ym_d = nc.dram_tensor("ym", [(NT - 1) * 2 * 128, FREE_Y // 2], bf16,
                          kind="ExternalOutput")
    ym3_d = nc.dram_tensor("ym3", [4 * 128, FREE_Y // 4], bf16,
                           kind="ExternalOutput")
    yr_d = nc.dram_tensor("yr", [128, NW], bf16, kind="ExternalOutput")

    with tile.TileContext(nc) as tc:
        with (
            tc.tile_pool(name="wp", bufs=1) as wp,
            tc.tile_pool(name="xp", bufs=4) as xp,
            tc.tile_pool(name="op", bufs=3) as op_,
            tc.tile_pool(name="rp", bufs=1) as rp,
            tc.tile_pool(name="pp", bufs=4, space="PSUM") as pp,
        ):
            def loop_body():
                # constants ride the ACT queue (store queue idle at
                # start); rump deps (sbands/biasr) lead so rump compute can
                # run during the main block loads
                xrt = rp.tile([128, B_PER_CORE * WP], bf16, tag="xr")
                nc.sync.dma_start(out=xrt, in_=xr_d[:, :])
                sbands = wp.tile([128, KW * C * RUMP_OUT], bf16, tag="sbands")
                nc.scalar.dma_start(out=sbands, in_=sbands_d[:, :])
                biasrt = wp.tile([128, 1], f32, tag="biasr")
                nc.scalar.dma_start(out=biasrt, in_=biasr_d[:, :])
                biast = wp.tile([128, C], f32, tag="bias")
                nc.scalar.dma_start(out=biast, in_=bias_d[:, :])
                bands0 = wp.tile([128, C * KW * 126], bf16, tag="bands0")
                nc.scalar.dma_start(out=bands0, in_=bands0_d[:, :])
                bands = wp.tile([128, C * KW * 124], bf16, tag="bands")
                nc.scalar.dma_start(out=bands, in_=bands_d[:, :])
                bands3 = wp.tile([128, C * KW * 126], bf16, tag="bands3")
                nc.scalar.dma_start(out=bands3, in_=bands3_d[:, :])
                bts = [bands0, bands, bands, bands3]

                # block 0 is laid out (c, p, img, wp) in DRAM: per-channel
                # chunks load as [128, 1032] contiguous pieces so channel 0's
                # matmuls can start ~1us in instead of waiting for 2 MB
                xm_c = xm_d.ap().rearrange("r (b w) -> (r b) w", b=C)
                CWB = B_PER_CORE * WP
                xts = []
                for t in range(NT):
                    xt = xp.tile([128, FREE_X], bf16, tag="x",
                                 name=f"x_{t}")
                    if t == 0:
                        for cc in range(C):
                            nc.sync.dma_start(
                                out=xt[:, cc * CWB:(cc + 1) * CWB],
                                in_=xm_c[cc * 128:(cc + 1) * 128, :])
                    else:
                        nc.sync.dma_start(out=xt,
                                          in_=xm_d[128 * t:128 * (t + 1), :])
                    xts.append(xt)

                def evac(idx, psum_t, out_view, bias_ap):
                    if idx % 2 == 0:
                        nc.scalar.activation(out=out_view, in_=psum_t,
                                             func=IDENT, bias=bias_ap,
                                             scale=1.0)
                    else:
                        nc.vector.tensor_scalar_add(out=out_view, in0=psum_t,
                                                    scalar1=bias_ap)

                # rump first: one (8c x 16 in-row) group, 10 matmuls warm
                # the PE while the big block loads stream in
                MR = C * RUMP_OUT
                if mode != "dma":
                    ort = rp.tile([128, NW], bf16, tag="or")
                    pr = pp.tile([MR, NW], f32, tag="ps", name="ps_r")
                    xrv = xrt[:, :].rearrange("p (i w) -> p i w",
                                              i=B_PER_CORE)
                    for j in range(KW):
                        lhsT = sbands[:, j * MR:(j + 1) * MR]
                        for img in range(B_PER_CORE):
                            nc.tensor.matmul(
                                pr[:, img * W:(img + 1) * W], lhsT,
                                xrv[:, img, j:j + W],
                                start=(j == 0), stop=(j == KW - 1))
                    if mode != "pe":
                        evac(0, pr, ort[0:MR, :], biasrt[0:MR, 0:1])
                    if mode == "full":
                        nc.scalar.dma_start(out=yr_d[:, :], in_=ort[:, :])

                for t in range(NT):
                    if mode == "dma":
                        if t < NT - 1:
                            for hh in range(2):
                                r0 = (2 * t + hh) * 128
                                nc.scalar.dma_start(
                                    out=ym_d[r0:r0 + 128, :],
                                    in_=xts[t][:, hh * (FREE_Y // 2):
                                               (hh + 1) * (FREE_Y // 2)])
                        else:
                            for qq in range(4):
                                nc.scalar.dma_start(
                                    out=ym3_d[qq * 128:(qq + 1) * 128, :],
                                    in_=xts[t][:, qq * (FREE_Y // 4):
                                               (qq + 1) * (FREE_Y // 4)])
                        continue
                    ot = op_.tile([128, FREE_Y], bf16, tag="o", name=f"o_{t}")
                    # [128, (c, img, wp)] view for strided rhs windows
                    xv = xts[t][:, :].rearrange("p (c i w) -> p c i w",
                                                c=C, i=B_PER_CORE)
                    MB = MTS[t]
                    for c in range(C):
                        pt = pp.tile([MB, NW], f32, tag="ps",
                                     name=f"ps_{t}_{c}")
                        for j in range(KW):
                            lhsT = bts[t][:, (c * KW + j) * MB:
                                          (c * KW + j + 1) * MB]
                            for img in range(B_PER_CORE):
                                nc.tensor.matmul(
                                    pt[:, img * W:(img + 1) * W], lhsT,
                                    xv[:, c, img, j:j + W],
                                    start=(j == 0), stop=(j == KW - 1))
                        if mode != "pe":
                            evac(c, pt, ot[0:MB, c * NW:(c + 1) * NW],
                                 biast[0:MB, c:c + 1])
                    if mode == "full":
                        if t < NT - 1:
                            for hh in range(2):
                                r0 = (2 * t + hh) * 128
                                nc.scalar.dma_start(
                                    out=ym_d[r0:r0 + 128, :],
                                    in_=ot[:, hh * (FREE_Y // 2):
                                           (hh + 1) * (FREE_Y // 2)])
                        else:
                            # last block: 4 quarter-stores on both queues so
                            # the drain tail is one 512 KB transfer
                            for qq in range(4):
                                eng = nc.scalar if qq % 2 == 0 else nc.sync
                                eng.dma_start(
                                    out=ym3_d[qq * 128:(qq + 1) * 128, :],
                                    in_=ot[:, qq * (FREE_Y // 4):
                                           (qq + 1) * (FREE_Y // 4)])

            if reps == 1:
                loop_body()
            else:
                with tc.For_i(0, reps, 1, hint_engines=(mybir.EngineType.PE,)):
                    loop_body()

    nc.compile()
    return nc


def _make_bands(K):
    """K: [5,5,C] (kh, kw, c) f32. Returns bf16 band matrices.

    Block bands map in-row partitions to out rows: out q uses in rows
    RSTART+q-2+d relative to the block. Edge blocks clip taps that fall
    outside [0,H) (those rows are SAME-padding zeros)."""
    bands0 = np.zeros((128, C * KW * 126), np.float32)
    bands12 = np.zeros((128, C * KW * 124), np.float32)
    bands3 = np.zeros((128, C * KW * 126), np.float32)
    for c in range(C):
        for j in range(KW):
            for d in range(KH):
                # block 0: partitions are in-rows 0..127, out q <- in q-2+d
                q = np.arange(126)
                p = q - 2 + d
                m = (p >= 0) & (p < 128)
                bands0[p[m], (c * KW + j) * 126 + q[m]] = K[d, j, c]
                # blocks 1,2: partitions in-rows base-2.., out q <- p=q+d
                q = np.arange(124)
                bands12[q + d, (c * KW + j) * 124 + q] = K[d, j, c]
                # block 3: partitions in-rows 384.., out 386+q <- p=q+d
                q = np.arange(126)
                p = q + d
                m = p < 128
                bands3[p[m], (c * KW + j) * 126 + q[m]] = K[d, j, c]
    # rump: partitions (c, hi) over in-rows 372..387; out 374+ho
    sbands = np.zeros((128, KW * C * RUMP_OUT), np.float32)
    ho = np.arange(RUMP_OUT)
    for j in range(KW):
        for c in range(C):
            for d in range(KH):
                sbands[c * RUMP_IN + ho + d,
                       j * C * RUMP_OUT + c * RUMP_OUT + ho] = K[d, j, c]
    return (bands0.astype(BF16), bands12.astype(BF16),
            bands3.astype(BF16), sbands.astype(BF16))


def prepare_in_maps(x, K, bias):
    """x [B,H,W,C] f32, K [5,5,C], bias [C] -> per-core input dicts."""
    bands0, bands12, bands3, sbands = _make_bands(K)
    bias128 = np.tile(bias[None, :], (128, 1)).astype(np.float32)
    biasr = np.zeros((128, 1), np.float32)
    for c in range(C):
        biasr[c * RUMP_OUT:(c + 1) * RUMP_OUT, 0] = bias[c]

    xb = x.astype(BF16)
    in_maps = []
    for i in range(N_CORES):
        xc = xb[i * B_PER_CORE:(i + 1) * B_PER_CORE]   # [2, 512, 512, 8]
        xh = np.zeros((B_PER_CORE, H + 4, C, WP), BF16)
        xh[:, 2:2 + H, :, PAD:PAD + W] = np.transpose(xc, (0, 1, 3, 2))
        xht = np.ascontiguousarray(xh.transpose(1, 2, 0, 3))  # [r, c, i, wp]
        # block t holds in-rows RSTART[t]..RSTART[t]+127 (r = in_row + 2)
        xm = np.empty((NT * 128, C, B_PER_CORE, WP), BF16)
        for t in range(NT):
            r0 = RSTART[t] + 2
            blk = xht[r0:r0 + 128]
            if t == 0:
                blk = np.ascontiguousarray(blk.transpose(1, 0, 2, 3))
            xm[128 * t:128 * (t + 1)] = blk.reshape(128, C, B_PER_CORE, WP)
        # rump: in-rows 372..387 -> r 374..390, packed (c, hi)
        xr = np.ascontiguousarray(
            xht[374:374 + RUMP_IN].transpose(1, 0, 2, 3)).reshape(
                128, B_PER_CORE * WP)
        in_maps.append({
            "xm": xm.reshape(NT * 128, FREE_X),
            "xr": xr,
            "bands0": bands0,
            "bands": bands12,
            "bands3": bands3,
            "sbands": sbands,
            "bias128": bias128,
            "biasr": biasr,
        })
    return in_maps


def assemble_output(results):
    """Per-core {ym, ym3, yr} bf16 -> full [B,H,W,C] f32."""
    out = np.empty((B, H, W, C), np.float32)
    for i in range(N_CORES):
        ym = np.asarray(results[i]["ym"]).reshape(NT - 1, 2, 128, C // 2,
                                                  B_PER_CORE, W)
        ym = ym.transpose(0, 2, 1, 3, 4, 5).reshape(NT - 1, 128, C,
                                                    B_PER_CORE, W)
        ym3 = np.asarray(results[i]["ym3"]).reshape(4, 128, C // 4,
                                                    B_PER_CORE, W)
        ym3 = ym3.transpose(1, 0, 2, 3, 4).reshape(128, C, B_PER_CORE, W)
        yr = np.asarray(results[i]["yr"]).reshape(128, B_PER_CORE, W)
        sl = slice(i * B_PER_CORE, (i + 1) * B_PER_CORE)
        for t in range(NT):
            blk = ym3 if t == NT - 1 else ym[t]
            # blk[q, c, img, w] -> out[img, OBASE+q, w, c]
            out[sl, OBASE[t]:OBASE[t] + MTS[t]] = \
                blk[:MTS[t]].transpose(2, 0, 3, 1).astype(np.float32)
        # rump rows 374..385: yr[(c, ho), img, w]
        yrr = yr[:C * RUMP_OUT].reshape(C, RUMP_OUT, B_PER_CORE, W)
        out[sl, 374:374 + RUMP_OUT] = \
            yrr.transpose(2, 1, 3, 0).astype(np.float32)
    return out


def kernel(x, kernel, bias):
    global _PROG, LAST_EXEC_NS
    from concourse.bass_utils import run_bass_kernel_spmd

    x = np.asarray(x, dtype=np.float32)
    K = np.asarray(kernel, dtype=np.float32).reshape(KH, KW, C)
    bias = np.asarray(bias, dtype=np.float32).reshape(C)

    if _PROG is None:
        _PROG = _build_program()

    in_maps = prepare_in_maps(x, K, bias)
    trace = os.environ.get("KERNEL_TRACE") == "1"
    res = run_bass_kernel_spmd(_PROG, in_maps, list(range(N_CORES)),
                               trace=trace)
    LAST_EXEC_NS = res.exec_time_ns
    if trace and res.exec_time_ns is not None:
        print(f"HW exec time: {res.exec_time_ns} ns")
    return assemble_output(res.results)
